# revision 18
# baseline (speedup 1.0000x reference)
"""Trainium2 Bass kernel for 16-head MultiHeadAttention (B=2, T=2048, D=1024).

Sharding (8 NeuronCores): core c handles batch b = c//4 and head group
g = c%4 (heads 4g..4g+3).  Each core computes Q/K/V projections for its 4
heads, attention, and a partial output projection against its 256 rows of
W_O.  The host sums the 4 partials per batch and adds b_O (row-parallel TP;
the all-reduce is folded into the unshard step).

Device layout notes:
 - The host pre-transposes x to x^T [D, T] so the contraction dim (features)
   lands on SBUF partitions without any on-device transposes of x.
 - Attention is computed in the S^T = K @ Q^T orientation: the softmax
   denominator is then a partition-axis sum, which the PE produces for free
   via a ones-column appended to V (out = [V|1]^T @ P^T gives O^T rows 0..63
   and the denominator in row 64).
 - V^T is produced directly in [s, dh] orientation by swapping matmul
   operand roles (stationary = x_from^T chunk, moving = Wv), with the bias
   AND the ones-columns injected by one extra K=1 matmul against an
   augmented bias row.  No PE transposes, no ACT copies.
 - The scalar engine runs ONLY the exp stream; everything else lives on
   DVE/Pool so ACT stays at its roofline.
 - Softmax reciprocal uses the fast custom-DVE approx (~5x faster than the
   table-based InstReciprocal) on the [1, 512] denominator rows.
 - Projections and the output projection are interleaved into the attention
   stripes as filler thunks so the PE never idles and stays at high pstate.
"""

import os
import sys

from collections import deque

import numpy as np

for _p in ("/opt/trn_rl_repo", "/root/.axon_site/_ro/trn_rl_repo"):
    if os.path.isdir(_p) and _p not in sys.path:
        sys.path.insert(0, _p)

import concourse.bass as bass
import concourse.mybir as mybir
import concourse.tile as tile
from concourse import bacc
from concourse.bass_utils import run_bass_kernel_spmd

F32 = mybir.dt.float32
BF16 = mybir.dt.bfloat16
FP8 = mybir.dt.float8e4
AF = mybir.ActivationFunctionType
DR = mybir.MatmulPerfMode.DoubleRow

B, TQ, TK = 2, 2048, 2048
D = 1024          # model dim == x_to/x_from feature dim
H, DH = 16, 64
N_CORES = 8
HEADS_PER_CORE = 4   # one batch per core
HP = 2               # head pairs per core (2 heads of 64 stacked -> 128)

TA = 512             # stripe width (queries per stripe)
N_SC = TK // 128     # 16 s-chunks
N_FC = D // 128      # 8 f-chunks
N_TT = 2             # q/k tiles of 1024 along t

DT = BF16

# fp8 score-path scales (powers of two, folded into Wq/Wk host-side and
# cancelled by the exp input scale)
A_Q = 32.0
A_K = 4.0
EXP_SCALE = 1.0 / (A_Q * A_K)

_CACHED = {}


def build_program():
    nc = bacc.Bacc(
        "TRN2", target_bir_lowering=False, debug=False, num_devices=N_CORES
    )

    xt_to = nc.dram_tensor("xt_to", [D, TQ], DT, kind="ExternalInput")
    xt_from = nc.dram_tensor("xt_from", [D, TK], DT, kind="ExternalInput")
    wq = nc.dram_tensor("wq", [D, 256], DT, kind="ExternalInput")
    wk = nc.dram_tensor("wk", [D, 256], DT, kind="ExternalInput")
    wv = nc.dram_tensor("wv", [D, 260], DT, kind="ExternalInput")
    bq = nc.dram_tensor("bq", [128, 2], F32, kind="ExternalInput")
    bk = nc.dram_tensor("bk", [128, 2], F32, kind="ExternalInput")
    bv = nc.dram_tensor("bv", [1, 260], DT, kind="ExternalInput")
    wot = nc.dram_tensor("wot", [128, 2, 1024], DT, kind="ExternalInput")
    out = nc.dram_tensor("out", [TQ, D], F32, kind="ExternalOutput")

    with tile.TileContext(nc) as tc:
        with (
            tc.tile_pool(name="wpool", bufs=1) as wpool,
            tc.tile_pool(name="actpool", bufs=1) as actpool,
            tc.tile_pool(name="ptpool", bufs=3) as ptpool,
            tc.tile_pool(name="misc", bufs=2) as misc,
            tc.tile_pool(name="psmm", bufs=2, space="PSUM") as psmm,
            tc.tile_pool(name="psacc", bufs=2, space="PSUM") as psacc,
            tc.tile_pool(name="psaux", bufs=2, space="PSUM") as psaux,
        ):
            # ---- weights / constants -------------------------------------
            wq_sb = wpool.tile([128, N_FC, 256], DT)
            wk_sb = wpool.tile([128, N_FC, 256], DT)
            wv_sb = wpool.tile([128, N_FC, 260], DT)
            bq_sb = wpool.tile([128, 2], F32)
            bk_sb = wpool.tile([128, 2], F32)
            bv_sb = wpool.tile([1, 260], DT)
            wot_sb = wpool.tile([128, 2, 1024], DT)
            ones_sb = wpool.tile([1, 128], DT)
            nc.vector.memset(ones_sb[:], 1.0)

            # x^T resident in SBUF, per-f-chunk DMAs
            xfr_sb = actpool.tile([128, N_FC, TK], DT, name="xfr_sb")
            xto_sb = actpool.tile([128, N_FC, TQ], DT, name="xto_sb")
            xt_to_r = xt_to.rearrange("(c p) t -> p c t", p=128)
            xt_from_r = xt_from.rearrange("(c p) t -> p c t", p=128)

            # DMA priority: K/V inputs and weights first so the prologue
            # projections can start as soon as possible.
            nc.sync.dma_start(wk_sb[:], wk.rearrange("(c p) d -> p c d", p=128))
            nc.sync.dma_start(wv_sb[:], wv.rearrange("(c p) d -> p c d", p=128))
            nc.sync.dma_start(bv_sb[:], bv[:])
            nc.sync.dma_start(bk_sb[:], bk[:])
            for fc in range(N_FC):
                nc.sync.dma_start(xfr_sb[:, fc, :], xt_from_r[:, fc, :])
            nc.sync.dma_start(wq_sb[:], wq.rearrange("(c p) d -> p c d", p=128))
            nc.sync.dma_start(bq_sb[:], bq[:])
            for fc in range(N_FC):
                nc.sync.dma_start(xto_sb[:, fc, :], xt_to_r[:, fc, :])
            nc.sync.dma_start(wot_sb[:], wot[:])

            # ---- persistent activations ----------------------------------
            # Q/K in fp8 DoubleRow layout: [32-partition head rows, 2 d-half
            # pairs, t].  Partition p (0..63), pair j holds head p//32,
            # d = 32*j + p%32 (the projection weight columns are permuted
            # host-side so the psum partitions land directly in this layout)
            qt_sb = [
                actpool.tile([64, 2, TQ], FP8, name=f"qt{hp}")
                for hp in range(HP)
            ]
            kt_sb = [
                actpool.tile([64, 2, TK], FP8, name=f"kt{hp}")
                for hp in range(HP)
            ]
            # V^T with ones columns: head h at cols 65h..65h+63, ones at
            # 65h+64 (4 heads -> 260 cols), per 128-wide s-chunk
            vn_sb = actpool.tile([128, N_SC, 260], DT, name="vn_sb")
            ot_sb = [
                actpool.tile([128, TQ], DT, name=f"ot{hp}") for hp in range(HP)
            ]

            # ---- thunk emitters ------------------------------------------
            def qk_thunks(w_sb, b_sb, x_sb, dst, hp, tt):
                """Q/K projection for one [128, 1024] tile: two psum halves,
                each 8 accumulating matmuls + a bias-add copyback."""
                thunks = []
                dsl = bass.ts(hp, 128)
                for half in range(2):
                    ps = psaux.tile([128, 512], F32, name="ps_x")
                    t0 = tt * 1024 + half * 512
                    for fcp in range(N_FC // 2):
                        def fill(fcp=fcp, ps=ps, t0=t0):
                            for fc in (2 * fcp, 2 * fcp + 1):
                                nc.tensor.matmul(
                                    ps[:],
                                    w_sb[:, fc, dsl],
                                    x_sb[:, fc, t0:t0 + 512],
                                    start=(fc == 0),
                                    stop=(fc == N_FC - 1),
                                )
                        thunks.append(fill)

                    def copyback(ps=ps, t0=t0):
                        # psum partitions 0..63 = pair j=0, 64..127 = j=1
                        for j in range(2):
                            nc.vector.tensor_scalar_add(
                                dst[hp][0:64, j, t0:t0 + 512],
                                ps[64 * j:64 * j + 64, :],
                                b_sb[64 * j:64 * j + 64, hp:hp + 1],
                            )
                    thunks.append(copyback)
                return thunks

            def v_thunks(sc):
                """V^T for one s-chunk, computed directly in [s, dh]
                orientation: stationary = x_from^T chunk, moving = Wv.
                Bias + ones columns injected via a K=1 matmul."""
                thunks = []
                ps = psaux.tile([128, 512], F32, name="ps_x")
                ssl = bass.ts(sc, 128)
                for fcp in range(N_FC // 2):
                    def fill(fcp=fcp, ps=ps):
                        for fc in (2 * fcp, 2 * fcp + 1):
                            nc.tensor.matmul(
                                ps[:, 0:260],
                                xfr_sb[:, fc, ssl],
                                wv_sb[:, fc, :],
                                start=(fc == 0),
                                stop=False,
                            )
                    thunks.append(fill)

                def bias(ps=ps):
                    nc.tensor.matmul(
                        ps[:, 0:260],
                        ones_sb[:],
                        bv_sb[:],
                        start=False,
                        stop=True,
                    )
                thunks.append(bias)

                def copyback(ps=ps):
                    nc.vector.tensor_copy(vn_sb[:, sc, :], ps[:, 0:260])
                thunks.append(copyback)
                return thunks

            def outproj_thunks(tta):
                """Output projection for one stripe of queries: 4 t-chunks
                of 128, each = 2 psum halves (contraction over both head
                pairs) + copyback, then one DMA."""
                thunks = []
                for j in range(TA // 128):
                    tc_ = tta * (TA // 128) + j
                    tsl = bass.ts(tc_, 128)
                    o_t = misc.tile([128, 1024], F32, name="o_t")
                    for half in range(2):
                        ps = psaux.tile([128, 512], F32, name="ps_x")
                        hsl = bass.ts(half, 512)

                        def mmf(ps=ps, tsl=tsl, hsl=hsl):
                            for hp in range(HP):
                                nc.tensor.matmul(
                                    ps[:],
                                    ot_sb[hp][:, tsl],
                                    wot_sb[:, hp, hsl],
                                    start=(hp == 0),
                                    stop=(hp == HP - 1),
                                )
                        thunks.append(mmf)

                        def cb(ps=ps, o_t=o_t, hsl=hsl):
                            nc.vector.tensor_copy(o_t[:, hsl], ps[:])
                        thunks.append(cb)

                    def store(o_t=o_t, tsl=tsl):
                        nc.sync.dma_start(out[tsl, :], o_t[:])
                    thunks.append(store)
                return thunks

            def emit_stripe(tta, hp, fillers, per_iter):
                """One attention stripe: both heads of the pair, 512
                queries, all 2048 keys.  Pops filler thunks per s-chunk so
                independent PE work interleaves with the ACT exp stream.
                ps_o is split per head on a bufs=2 ring so the next
                stripe's PV only waits on the matching head's drain."""
                ps_o = [
                    psacc.tile([65, TA], F32, name="ps_o") for _ in range(2)
                ]
                for sc in range(N_SC):
                    ps_s = psmm.tile([128, 1024], F32, name="ps_s")
                    for h in range(2):
                        hb = 32 * h
                        nc.tensor.matmul(
                            ps_s[:, bass.ts(h, TA)],
                            kt_sb[hp][hb:hb + 32, :, bass.ts(sc, 128)],
                            qt_sb[hp][hb:hb + 32, :, bass.ts(tta, TA)],
                            start=True,
                            stop=True,
                            perf_mode=DR,
                        )
                    pt = ptpool.tile([128, 1024], DT, name="pt")
                    nc.scalar.activation(pt[:], ps_s[:], AF.Exp, scale=EXP_SCALE)
                    for h in range(2):
                        vb = 65 * (2 * hp + h)
                        nc.tensor.matmul(
                            ps_o[h][:],
                            vn_sb[:, sc, vb:vb + 65],
                            pt[:, bass.ts(h, TA)],
                            start=(sc == 0),
                            stop=(sc == N_SC - 1),
                        )
                    for _ in range(per_iter):
                        if fillers:
                            fillers.popleft()()

                # denominators (psum row 64) -> fast reciprocal -> broadcast
                # across partitions (Pool) -> normalize ps_o into ot (DVE)
                # (reciprocal_approx_fast must NOT read PSUM directly)
                recs = []
                for h in range(2):
                    rec = misc.tile([1, TA], F32, name="rec_t")
                    nc.vector.tensor_copy(rec[:], ps_o[h][64:65, :])
                    nc.vector.reciprocal_approx_fast(rec[:], rec[:])
                    recs.append(rec)
                for h in range(2):
                    r_sb = misc.tile([128, TA], F32, name="r_sb")
                    nc.gpsimd.partition_broadcast(r_sb[:], recs[h][:])
                    hb = 64 * h
                    nc.vector.tensor_mul(
                        ot_sb[hp][hb:hb + 64, bass.ts(tta, TA)],
                        ps_o[h][0:64, :],
                        r_sb[0:64, :],
                    )

            # ---- emission schedule ---------------------------------------
            # Minimal prologue so the exp stream starts as early as
            # possible: K(hp0,tt0) covers keys for s-chunks 0..7, V(0..7),
            # Q(hp0,tt0).  The rest (K tt1, V 8..15, the other head pair,
            # remaining Q tiles, output projections) streams in as filler
            # thunks inside the stripes.  A stripe's own s-chunk loop only
            # consumes V(sc)/K(tt1) at iteration sc >= 8, by which point the
            # high filler rate of the first stripe has emitted them.
            # Everything a stripe needs at its FIRST iteration must be fully
            # emitted before the stripe starts (guarded by drain counts).
            for f in qk_thunks(wk_sb, bk_sb, xfr_sb, kt_sb, 0, 0):
                f()
            for sc in range(8):
                for f in v_thunks(sc):
                    f()
            for f in qk_thunks(wq_sb, bq_sb, xto_sb, qt_sb, 0, 0):
                f()

            fillers = deque()
            fillers.extend(qk_thunks(wk_sb, bk_sb, xfr_sb, kt_sb, 0, 1))
            for sc in range(8, N_SC):
                fillers.extend(v_thunks(sc))
            # needed before stripe (0, 1):
            fillers.extend(qk_thunks(wk_sb, bk_sb, xfr_sb, kt_sb, 1, 0))
            fillers.extend(qk_thunks(wk_sb, bk_sb, xfr_sb, kt_sb, 1, 1))
            fillers.extend(qk_thunks(wq_sb, bq_sb, xto_sb, qt_sb, 1, 0))
            n_before_01 = len(fillers)
            # needed before stripes (1, 0) / (1, 1):
            fillers.extend(qk_thunks(wq_sb, bq_sb, xto_sb, qt_sb, 0, 1))
            n_before_10 = len(fillers)
            fillers.extend(qk_thunks(wq_sb, bq_sb, xto_sb, qt_sb, 1, 1))
            n_before_11 = len(fillers)
            popped = [0]

            def drain_to(target):
                while fillers and popped[0] < target:
                    fillers.popleft()()
                    popped[0] += 1

            def pop_n(n):
                for _ in range(n):
                    if fillers:
                        fillers.popleft()()
                        popped[0] += 1

            per_iter_schedule = {
                (0, 0): 7, (0, 1): 3, (1, 0): 3, (1, 1): 2,
                (2, 0): 2, (2, 1): 2, (3, 0): 2, (3, 1): 2,
            }
            guards = {(0, 1): n_before_01, (1, 0): n_before_10,
                      (1, 1): n_before_11}

            class _Pop:
                def __init__(self):
                    self.q = fillers

                def popleft(self):
                    popped[0] += 1
                    return fillers.popleft()

                def __bool__(self):
                    return bool(fillers)

                def __len__(self):
                    return len(fillers)

            popper = _Pop()
            for tta in range(TQ // TA):
                for hp in range(HP):
                    drain_to(guards.get((tta, hp), 0))
                    emit_stripe(tta, hp, popper, per_iter_schedule[(tta, hp)])
                fillers.extend(outproj_thunks(tta))

            while fillers:
                fillers.popleft()()

    nc.compile()
    return nc


def _prep_in_maps(x_to, x_from, Wq, bq, Wk, bk, Wv, bv, Wo):
    scale = 1.0 / np.sqrt(np.float32(DH))
    # [H, D, DH] -> [D, H*DH] with column h*DH+d ; fp8 score scales folded
    wq_f = (
        np.ascontiguousarray(Wq.transpose(1, 0, 2).reshape(D, H * DH))
        * (scale * A_Q)
    )
    wk_f = np.ascontiguousarray(Wk.transpose(1, 0, 2).reshape(D, H * DH)) * A_K
    bq_f = bq.reshape(H * DH) * (scale * A_Q)
    bk_f = bk.reshape(H * DH) * A_K

    # permutation of each 128-wide head-pair block so the projection psum
    # partitions land directly in the fp8 DoubleRow [32-rows, 2-pair] layout:
    # new col p <- orig col 64*hh + 32*j + dd  (hh=(p%64)//32, j=p//64, dd=p%32)
    perm128 = np.empty(128, dtype=np.int64)
    for p in range(128):
        hh, j, dd = (p % 64) // 32, p // 64, p % 32
        perm128[p] = 64 * hh + 32 * j + dd
    perm256 = np.concatenate([perm128, 128 + perm128])

    xt_to = np.ascontiguousarray(x_to.transpose(0, 2, 1))    # [B, D, TQ]
    xt_from = np.ascontiguousarray(x_from.transpose(0, 2, 1))

    def f32(a):
        return np.ascontiguousarray(a, dtype=np.float32)

    import ml_dtypes

    def fdt(a):
        return np.ascontiguousarray(a, dtype=ml_dtypes.bfloat16)

    in_maps = []
    for c in range(N_CORES):
        b, g = divmod(c, HEADS_PER_CORE)
        cs = slice(g * 256, (g + 1) * 256)
        # Wv augmented: head h (of the core's 4) at cols 65h..65h+63,
        # zero col at 65h+64; bias row gets bv there plus 1.0 ones
        wv_aug = np.zeros((D, 260), dtype=np.float32)
        bv_aug = np.zeros((260,), dtype=np.float32)
        for h in range(4):
            head = 4 * g + h
            wv_aug[:, 65 * h:65 * h + 64] = Wv[head]
            bv_aug[65 * h:65 * h + 64] = bv[head]
            bv_aug[65 * h + 64] = 1.0
        in_maps.append(
            {
                "xt_to": fdt(xt_to[b]),
                "xt_from": fdt(xt_from[b]),
                "wq": fdt(wq_f[:, cs][:, perm256]),
                "wk": fdt(wk_f[:, cs][:, perm256]),
                "wv": fdt(wv_aug),
                # [256] -> [2 pairs, 128] -> [128, 2], rows permuted to match
                "bq": f32(bq_f[cs][perm256].reshape(2, 128).T),
                "bk": f32(bk_f[cs][perm256].reshape(2, 128).T),
                "bv": fdt(bv_aug.reshape(1, 260)),
                # Wo[:, cs].T = [256, 1024] -> [2, 128, 1024] -> [128, 2, 1024]
                "wot": fdt(
                    np.ascontiguousarray(Wo[:, cs].T)
                    .reshape(2, 128, 1024)
                    .transpose(1, 0, 2)
                ),
            }
        )
    return in_maps


LAST_EXEC_TIME_NS = None
LAST_TRACE = None


def kernel(x_to, x_from, Wq, bq, Wk, bk, Wv, bv, Wo, bo):
    global LAST_EXEC_TIME_NS, LAST_TRACE
    if "nc" not in _CACHED:
        _CACHED["nc"] = build_program()
    nc = _CACHED["nc"]

    in_maps = _prep_in_maps(
        np.asarray(x_to), np.asarray(x_from), np.asarray(Wq), np.asarray(bq),
        np.asarray(Wk), np.asarray(bk), np.asarray(Wv), np.asarray(bv),
        np.asarray(Wo),
    )
    res = run_bass_kernel_spmd(nc, in_maps, list(range(N_CORES)))
    LAST_EXEC_TIME_NS = res.exec_time_ns
    LAST_TRACE = res.instructions_and_trace

    out = np.zeros((B, TQ, D), dtype=np.float32)
    for c in range(N_CORES):
        out[c // HEADS_PER_CORE] += res.results[c]["out"]
    out += np.asarray(bo, dtype=np.float32)
    return out


# revision 19
# speedup vs baseline: 1.0585x; 1.0585x over previous
"""Trainium2 Bass kernel for 16-head MultiHeadAttention (B=2, T=2048, D=1024).

Sharding (8 NeuronCores): core c handles batch b = c//4 and head group
g = c%4 (heads 4g..4g+3).  Each core computes Q/K/V projections for its 4
heads, attention, and a partial output projection against its 256 rows of
W_O.  The host sums the 4 partials per batch and adds b_O (row-parallel TP;
the all-reduce is folded into the unshard step).

Device layout notes:
 - The host pre-transposes x to x^T [D, T] so the contraction dim (features)
   lands on SBUF partitions without any on-device transposes of x.  The 8
   128-row feature chunks are separate SBUF tiles so projection matmuls can
   start as soon as the first chunk's DMA lands.
 - Attention is computed in the S^T = K @ Q^T orientation: the softmax
   denominator is then a partition-axis sum, which the PE produces for free
   via a ones-column appended to V (out = [V|1]^T @ P^T gives O^T rows 0..63
   and the denominator in row 64).
 - V^T is produced directly in [s, dh] orientation by swapping matmul
   operand roles (stationary = x_from^T chunk, moving = Wv), with the bias
   AND the ones-columns injected by one extra K=1 matmul against an
   augmented bias row.  No PE transposes, no ACT copies.
 - The scalar engine runs ONLY the exp stream; everything else lives on
   DVE/Pool so ACT stays at its roofline.
 - Softmax reciprocal uses the fast custom-DVE approx (~5x faster than the
   table-based InstReciprocal) on the [1, 512] denominator rows.
 - Projections and the output projection are interleaved into the attention
   stripes as filler thunks so the PE never idles and stays at high pstate.
   The prologue is minimal (half of K/V, one Q tile) so the exp stream
   starts as early as the input DMA allows.
"""

import os
import sys

from collections import deque

import numpy as np

for _p in ("/opt/trn_rl_repo", "/root/.axon_site/_ro/trn_rl_repo"):
    if os.path.isdir(_p) and _p not in sys.path:
        sys.path.insert(0, _p)

import concourse.bass as bass
import concourse.mybir as mybir
import concourse.tile as tile
from concourse import bacc
from concourse.bass_utils import run_bass_kernel_spmd

F32 = mybir.dt.float32
BF16 = mybir.dt.bfloat16
AF = mybir.ActivationFunctionType

B, TQ, TK = 2, 2048, 2048
D = 1024          # model dim == x_to/x_from feature dim
H, DH = 16, 64
N_CORES = 8
HEADS_PER_CORE = 4   # one batch per core
HP = 2               # head pairs per core (2 heads of 64 stacked -> 128)

TA = 512             # stripe width (queries per stripe)
N_SC = TK // 128     # 16 s-chunks
N_FC = D // 128      # 8 f-chunks

DT = BF16

_CACHED = {}


def build_program():
    nc = bacc.Bacc(
        "TRN2", target_bir_lowering=False, debug=False, num_devices=N_CORES
    )

    xt_to = nc.dram_tensor("xt_to", [D, TQ], DT, kind="ExternalInput")
    xt_from = nc.dram_tensor("xt_from", [D, TK], DT, kind="ExternalInput")
    wq = nc.dram_tensor("wq", [D, 256], DT, kind="ExternalInput")
    wk = nc.dram_tensor("wk", [D, 256], DT, kind="ExternalInput")
    wv = nc.dram_tensor("wv", [D, 260], DT, kind="ExternalInput")
    bq = nc.dram_tensor("bq", [128, 2], F32, kind="ExternalInput")
    bk = nc.dram_tensor("bk", [128, 2], F32, kind="ExternalInput")
    bv = nc.dram_tensor("bv", [1, 260], DT, kind="ExternalInput")
    wot = nc.dram_tensor("wot", [128, 2, 1024], DT, kind="ExternalInput")
    out = nc.dram_tensor("out", [TQ, D], DT, kind="ExternalOutput")

    with tile.TileContext(nc) as tc:
        with (
            tc.tile_pool(name="wpool", bufs=1) as wpool,
            tc.tile_pool(name="actpool", bufs=1) as actpool,
            tc.tile_pool(name="ptpool", bufs=3) as ptpool,
            tc.tile_pool(name="misc", bufs=2) as misc,
            tc.tile_pool(name="psmm", bufs=2, space="PSUM") as psmm,
            tc.tile_pool(name="psacc", bufs=2, space="PSUM") as psacc,
            tc.tile_pool(name="psaux", bufs=2, space="PSUM") as psaux,
        ):
            # ---- weights / constants -------------------------------------
            wq_sb = wpool.tile([128, N_FC, 256], DT)
            wk_sb = wpool.tile([128, N_FC, 256], DT)
            wv_sb = wpool.tile([128, N_FC, 260], DT)
            bq_sb = wpool.tile([128, 2], F32)
            bk_sb = wpool.tile([128, 2], F32)
            bv_sb = wpool.tile([1, 260], DT)
            wot_sb = wpool.tile([128, 2, 1024], DT)
            ones_sb = wpool.tile([1, 128], DT)
            nc.vector.memset(ones_sb[:], 1.0)

            # x^T chunks as separate tiles -> exact DMA->matmul deps
            xfr_sb = [
                actpool.tile([128, TK], DT, name=f"xfr{fc}")
                for fc in range(N_FC)
            ]
            xto_sb = [
                actpool.tile([128, TQ], DT, name=f"xto{fc}")
                for fc in range(N_FC)
            ]
            xt_to_r = xt_to.rearrange("(c p) t -> p c t", p=128)
            xt_from_r = xt_from.rearrange("(c p) t -> p c t", p=128)

            # DMA priority: K/V inputs and weights first so the prologue
            # projections can start as soon as possible.
            nc.sync.dma_start(wk_sb[:], wk.rearrange("(c p) d -> p c d", p=128))
            nc.sync.dma_start(wv_sb[:], wv.rearrange("(c p) d -> p c d", p=128))
            nc.sync.dma_start(bv_sb[:], bv[:])
            nc.sync.dma_start(bk_sb[:], bk[:])
            for fc in range(N_FC):
                nc.sync.dma_start(xfr_sb[fc][:], xt_from_r[:, fc, :])
            nc.sync.dma_start(wq_sb[:], wq.rearrange("(c p) d -> p c d", p=128))
            nc.sync.dma_start(bq_sb[:], bq[:])
            for fc in range(N_FC):
                nc.sync.dma_start(xto_sb[fc][:], xt_to_r[:, fc, :])
            nc.sync.dma_start(wot_sb[:], wot[:])

            # ---- persistent activations ----------------------------------
            qt_sb = [
                actpool.tile([128, TQ], DT, name=f"qt{hp}") for hp in range(HP)
            ]
            kt_sb = [
                actpool.tile([128, TK], DT, name=f"kt{hp}") for hp in range(HP)
            ]
            # V^T with ones columns: head h at cols 65h..65h+63, ones at
            # 65h+64 (4 heads -> 260 cols), per 128-wide s-chunk
            vn_sb = actpool.tile([128, N_SC, 260], DT, name="vn_sb")
            ot_sb = [
                actpool.tile([128, TQ], DT, name=f"ot{hp}") for hp in range(HP)
            ]

            # ---- thunk emitters ------------------------------------------
            def qk_thunks(w_sb, b_sb, x_sb, dst, hp, tt):
                """Q/K projection for one [128, 1024] tile: two psum halves,
                each 8 accumulating matmuls + a bias-add copyback."""
                thunks = []
                dsl = bass.ts(hp, 128)
                for half in range(2):
                    ps = psaux.tile([128, 512], F32, name="ps_x")
                    t0 = tt * 1024 + half * 512
                    for fcp in range(N_FC // 2):
                        def fill(fcp=fcp, ps=ps, t0=t0):
                            for fc in (2 * fcp, 2 * fcp + 1):
                                nc.tensor.matmul(
                                    ps[:],
                                    w_sb[:, fc, dsl],
                                    x_sb[fc][:, t0:t0 + 512],
                                    start=(fc == 0),
                                    stop=(fc == N_FC - 1),
                                )
                        thunks.append(fill)

                    def copyback(ps=ps, t0=t0):
                        nc.vector.tensor_scalar_add(
                            dst[hp][:, t0:t0 + 512], ps[:], b_sb[:, hp:hp + 1]
                        )
                    thunks.append(copyback)
                return thunks

            def v_thunks(sc):
                """V^T for one s-chunk, computed directly in [s, dh]
                orientation: stationary = x_from^T chunk, moving = Wv.
                Bias + ones columns injected via a K=1 matmul."""
                thunks = []
                ps = psaux.tile([128, 512], F32, name="ps_x")
                ssl = bass.ts(sc, 128)
                for fcp in range(N_FC // 2):
                    def fill(fcp=fcp, ps=ps):
                        for fc in (2 * fcp, 2 * fcp + 1):
                            nc.tensor.matmul(
                                ps[:, 0:260],
                                xfr_sb[fc][:, ssl],
                                wv_sb[:, fc, :],
                                start=(fc == 0),
                                stop=False,
                            )
                    thunks.append(fill)

                def bias(ps=ps):
                    nc.tensor.matmul(
                        ps[:, 0:260],
                        ones_sb[:],
                        bv_sb[:],
                        start=False,
                        stop=True,
                    )
                thunks.append(bias)

                def copyback(ps=ps):
                    nc.vector.tensor_copy(vn_sb[:, sc, :], ps[:, 0:260])
                thunks.append(copyback)
                return thunks

            def outproj_thunks(tta):
                """Output projection for one stripe of queries: 4 t-chunks
                of 128, each = 2 psum halves (contraction over both head
                pairs) + copyback, then one DMA."""
                thunks = []
                for j in range(TA // 128):
                    tc_ = tta * (TA // 128) + j
                    tsl = bass.ts(tc_, 128)
                    o_t = misc.tile([128, 1024], DT, name="o_t")
                    for half in range(2):
                        ps = psaux.tile([128, 512], F32, name="ps_x")
                        hsl = bass.ts(half, 512)

                        def mmf(ps=ps, tsl=tsl, hsl=hsl):
                            for hp in range(HP):
                                nc.tensor.matmul(
                                    ps[:],
                                    ot_sb[hp][:, tsl],
                                    wot_sb[:, hp, hsl],
                                    start=(hp == 0),
                                    stop=(hp == HP - 1),
                                )
                        thunks.append(mmf)

                        def cb(ps=ps, o_t=o_t, hsl=hsl):
                            nc.vector.tensor_copy(o_t[:, hsl], ps[:])
                        thunks.append(cb)

                    def store(o_t=o_t, tsl=tsl):
                        nc.sync.dma_start(out[tsl, :], o_t[:])
                    thunks.append(store)
                return thunks

            def emit_stripe(tta, hp, pop_filler, per_iter):
                """One attention stripe: both heads of the pair, 512
                queries, all 2048 keys.  Pops filler thunks per s-chunk so
                independent PE work interleaves with the ACT exp stream.
                ps_o is split per head on a bufs=2 ring so the next
                stripe's PV only waits on the matching head's drain."""
                ps_o = [
                    psacc.tile([65, TA], F32, name="ps_o") for _ in range(2)
                ]
                for sc in range(N_SC):
                    ps_s = psmm.tile([128, 1024], F32, name="ps_s")
                    for h in range(2):
                        hb = 64 * h
                        nc.tensor.matmul(
                            ps_s[:, bass.ts(h, TA)],
                            kt_sb[hp][hb:hb + 64, bass.ts(sc, 128)],
                            qt_sb[hp][hb:hb + 64, bass.ts(tta, TA)],
                            start=True,
                            stop=True,
                        )
                    pt = ptpool.tile([128, 1024], DT, name="pt")
                    nc.scalar.activation(pt[:], ps_s[:], AF.Exp)
                    for h in range(2):
                        vb = 65 * (2 * hp + h)
                        nc.tensor.matmul(
                            ps_o[h][:],
                            vn_sb[:, sc, vb:vb + 65],
                            pt[:, bass.ts(h, TA)],
                            start=(sc == 0),
                            stop=(sc == N_SC - 1),
                        )
                    pop_filler(per_iter)

                # denominators (psum row 64) -> fast reciprocal -> broadcast
                # across partitions (Pool) -> normalize ps_o into ot (DVE)
                # (reciprocal_approx_fast must NOT read PSUM directly)
                recs = []
                for h in range(2):
                    rec = misc.tile([1, TA], F32, name="rec_t")
                    nc.vector.tensor_copy(rec[:], ps_o[h][64:65, :])
                    nc.vector.reciprocal_approx_fast(rec[:], rec[:])
                    recs.append(rec)
                for h in range(2):
                    r_sb = misc.tile([128, TA], F32, name="r_sb")
                    nc.gpsimd.partition_broadcast(r_sb[:], recs[h][:])
                    hb = 64 * h
                    nc.vector.tensor_mul(
                        ot_sb[hp][hb:hb + 64, bass.ts(tta, TA)],
                        ps_o[h][0:64, :],
                        r_sb[0:64, :],
                    )

            # ---- emission schedule ---------------------------------------
            # Minimal prologue so the exp stream starts as early as the
            # input DMA allows: K(hp0,tt0) covers keys for s-chunks 0..7,
            # V(0..7), Q(hp0,tt0).  Everything else (K tt1, V 8..15, the
            # other head pair, remaining Q tiles, output projections)
            # streams in as filler thunks inside the stripes.  A stripe's
            # own s-chunk loop only consumes V(sc)/K(tt1) at iteration
            # sc >= 8, by which point the first stripe's high filler rate
            # has emitted them.  Anything a stripe needs at its FIRST
            # iteration is force-drained before the stripe starts.
            for f in qk_thunks(wk_sb, bk_sb, xfr_sb, kt_sb, 0, 0):
                f()
            for sc in range(8):
                for f in v_thunks(sc):
                    f()
            for f in qk_thunks(wq_sb, bq_sb, xto_sb, qt_sb, 0, 0):
                f()

            fillers = deque()
            fillers.extend(qk_thunks(wk_sb, bk_sb, xfr_sb, kt_sb, 0, 1))
            for sc in range(8, N_SC):
                fillers.extend(v_thunks(sc))
            fillers.extend(qk_thunks(wk_sb, bk_sb, xfr_sb, kt_sb, 1, 0))
            fillers.extend(qk_thunks(wk_sb, bk_sb, xfr_sb, kt_sb, 1, 1))
            fillers.extend(qk_thunks(wq_sb, bq_sb, xto_sb, qt_sb, 1, 0))
            n_before_01 = len(fillers)
            fillers.extend(qk_thunks(wq_sb, bq_sb, xto_sb, qt_sb, 0, 1))
            n_before_10 = len(fillers)
            fillers.extend(qk_thunks(wq_sb, bq_sb, xto_sb, qt_sb, 1, 1))
            n_before_11 = len(fillers)

            popped = [0]

            def pop_filler(n):
                for _ in range(n):
                    if fillers:
                        fillers.popleft()()
                        popped[0] += 1

            def drain_to(target):
                while fillers and popped[0] < target:
                    fillers.popleft()()
                    popped[0] += 1

            per_iter_schedule = {
                (0, 0): 7, (0, 1): 3, (1, 0): 3, (1, 1): 2,
                (2, 0): 2, (2, 1): 2, (3, 0): 2, (3, 1): 2,
            }
            guards = {(0, 1): n_before_01, (1, 0): n_before_10,
                      (1, 1): n_before_11}

            for tta in range(TQ // TA):
                for hp in range(HP):
                    drain_to(guards.get((tta, hp), 0))
                    emit_stripe(tta, hp, pop_filler,
                                per_iter_schedule[(tta, hp)])
                fillers.extend(outproj_thunks(tta))

            while fillers:
                fillers.popleft()()

    nc.compile()
    return nc


def _prep_in_maps(x_to, x_from, Wq, bq, Wk, bk, Wv, bv, Wo):
    scale = 1.0 / np.sqrt(np.float32(DH))
    # [H, D, DH] -> [D, H*DH] with column h*DH+d
    wq_f = np.ascontiguousarray(Wq.transpose(1, 0, 2).reshape(D, H * DH)) * scale
    wk_f = np.ascontiguousarray(Wk.transpose(1, 0, 2).reshape(D, H * DH))
    bq_f = bq.reshape(H * DH) * scale
    bk_f = bk.reshape(H * DH)

    xt_to = np.ascontiguousarray(x_to.transpose(0, 2, 1))    # [B, D, TQ]
    xt_from = np.ascontiguousarray(x_from.transpose(0, 2, 1))

    def f32(a):
        return np.ascontiguousarray(a, dtype=np.float32)

    import ml_dtypes

    def fdt(a):
        return np.ascontiguousarray(a, dtype=ml_dtypes.bfloat16)

    in_maps = []
    for c in range(N_CORES):
        b, g = divmod(c, HEADS_PER_CORE)
        cs = slice(g * 256, (g + 1) * 256)
        # Wv augmented: head h (of the core's 4) at cols 65h..65h+63,
        # zero col at 65h+64; bias row gets bv there plus 1.0 ones
        wv_aug = np.zeros((D, 260), dtype=np.float32)
        bv_aug = np.zeros((260,), dtype=np.float32)
        for h in range(4):
            head = 4 * g + h
            wv_aug[:, 65 * h:65 * h + 64] = Wv[head]
            bv_aug[65 * h:65 * h + 64] = bv[head]
            bv_aug[65 * h + 64] = 1.0
        in_maps.append(
            {
                "xt_to": fdt(xt_to[b]),
                "xt_from": fdt(xt_from[b]),
                "wq": fdt(wq_f[:, cs]),
                "wk": fdt(wk_f[:, cs]),
                "wv": fdt(wv_aug),
                # [256] -> [2 pairs, 128] -> [128, 2]
                "bq": f32(bq_f[cs].reshape(2, 128).T),
                "bk": f32(bk_f[cs].reshape(2, 128).T),
                "bv": fdt(bv_aug.reshape(1, 260)),
                # Wo[:, cs].T = [256, 1024] -> [2, 128, 1024] -> [128, 2, 1024]
                "wot": fdt(
                    np.ascontiguousarray(Wo[:, cs].T)
                    .reshape(2, 128, 1024)
                    .transpose(1, 0, 2)
                ),
            }
        )
    return in_maps


LAST_EXEC_TIME_NS = None
LAST_TRACE = None


def kernel(x_to, x_from, Wq, bq, Wk, bk, Wv, bv, Wo, bo):
    global LAST_EXEC_TIME_NS, LAST_TRACE
    if "nc" not in _CACHED:
        _CACHED["nc"] = build_program()
    nc = _CACHED["nc"]

    in_maps = _prep_in_maps(
        np.asarray(x_to), np.asarray(x_from), np.asarray(Wq), np.asarray(bq),
        np.asarray(Wk), np.asarray(bk), np.asarray(Wv), np.asarray(bv),
        np.asarray(Wo),
    )
    res = run_bass_kernel_spmd(nc, in_maps, list(range(N_CORES)))
    LAST_EXEC_TIME_NS = res.exec_time_ns
    LAST_TRACE = res.instructions_and_trace

    out = np.zeros((B, TQ, D), dtype=np.float32)
    for c in range(N_CORES):
        out[c // HEADS_PER_CORE] += np.asarray(
            res.results[c]["out"], dtype=np.float32
        )
    out += np.asarray(bo, dtype=np.float32)
    return out
